# revision 8
# baseline (speedup 1.0000x reference)
"""Trainium2 Bass kernel for nn_ActorNetSpiking (4-layer spiking actor net), v2.

Strategy
--------
Data-parallel over batch: 8 NeuronCores x 512 rows each, [feature, batch]
layout on chip (contraction on partitions, no transposes anywhere).

Matmul precision plan (chosen by measuring end-to-end rel-err of each
quantization against the real spike-flip cascade; gate is 2e-2):
  L1: fp16 hi + fp16 res of both W1 and x, 3 passes, all 50 steps (early
      noise is amplified ~2.5x per downstream spiking layer; L1 single-pass
      alone costs ~2.9e-2 rel err).
  L2: hi+res for steps t < T2SPLIT(=30), hi-only after (late-step noise
      cannot cascade far).  Switching the effective W changes the folded
      bias, so u2 is rebased once at the switch (d2 columns).
  L3, L4: fp16 hi only, all steps.
Measured HW rel err: 0.0143.  PE work drops from 368 to 232..296 matmuls
per step vs the hi/res-everywhere baseline, and the 4MB/step residual-
weight DMA streams disappear (all weights resident, ~64KB/partition).

Spiking recurrence in shifted form: b_eff is folded into thr/twob/u0
constants (complement spikes r = 1-s feed negated weights), and spikes are
emitted pre-scaled by VDECAY with stationaries carrying 1/VDECAY, so the
vr-state holds vd = VDECAY*vr and the neuron update needs no
scalar_tensor_tensor on the w-path:
    u'_t = 0.5 u'_{t-1} + psum_t          (op1, DVE STT, psum operand,
                                           htile-paired 1024-wide)
    w_t  = vd_{t-1} + u'_t                (op2, plain TT add: GpSimd-legal;
                                           paired; split DVE/GpSimd)
    r_t  = (w_t <= thr) * VDECAY   fp16   (op3, dual-op TS, GpSimd)
    vd_t = (w_t + twob) * r_t             (op4, DVE STT, deferred one pair,
                                           skipped on the final step)
neuronxcc rejects TensorScalarPtr (STT) on Pool - only TT/TS run there.
Engine busy per step: PE ~58us (sim) / DVE ~34us / Pool ~18-33us, vs the
baseline's DVE ~55us.  CoreSim cost-model span: 2.94ms (baseline 3.97ms).
"""

import sys

sys.path.insert(0, "/opt/trn_rl_repo")

import numpy as np

# ---- problem constants (hardcoded per contract) ----
B, S, T = 4096, 512, 50
H = 1024
A = 2
NCORES = 8
BS = B // NCORES          # 512 batch rows per core
P = 128                   # partitions
KT1 = S // P              # 4 k-tiles for layer 1
KT = H // P               # 8 k-tiles for layers 2-4
HT = H // P               # 8 h-tiles for layers 1-3
NB = BS                   # matmul free dim

CDECAY, VDECAY, VTH = 0.5, 0.75, 0.5
F16_MIN_NORMAL = 6.104e-5

T2SPLIT = 30           # L2 uses hi+res for t < T2SPLIT, hi-only after
OP2_DVE = (0,)         # op2 (TT add) runs on DVE for these jp pairs, GpSimd else
REPEAT = 1             # timing experiments only: repeat the scan in one NEFF

_CACHE = {}


def _f16pair(a):
    """a (fp32) -> (hi fp16, res fp16) with hi+res ~ a to ~2^-24 abs."""
    hi = a.astype(np.float16).astype(np.float32)
    hi[np.abs(a) < 2 * F16_MIN_NORMAL] = 0.0
    res = (a - hi).astype(np.float16)
    return hi.astype(np.float16), res


def _build_program():
    import concourse.mybir as mybir
    import concourse.tile as tile
    from concourse import bacc

    f32 = mybir.dt.float32
    f16 = mybir.dt.float16
    AOT = mybir.AluOpType

    nc = bacc.Bacc("TRN2", target_bir_lowering=False, debug=False)

    # ---- DRAM tensors (streamed tensors are partition-major contiguous) ----
    xd = nc.dram_tensor("x", (T, P, 2 * KT1 * NB), f16, kind="ExternalInput")
    w1d = nc.dram_tensor("w1", (P, 2 * KT1 * H), f16, kind="ExternalInput")
    w2d = nc.dram_tensor("w2", (P, 2 * KT * H), f16, kind="ExternalInput")
    w3d = nc.dram_tensor("w3", (P, KT * H), f16, kind="ExternalInput")
    w4d = nc.dram_tensor("w4", (P, KT * A), f16, kind="ExternalInput")
    u0d = nc.dram_tensor("u0", (3, P, HT * NB), f32, kind="ExternalInput")
    # thr/twob columns: [L1(8) | L2E(8) | L2L(8) | L3(8)] ; rebase d2 [+d|-d]
    thrd = nc.dram_tensor("thr", (P, 4 * HT), f32, kind="ExternalInput")
    twobd = nc.dram_tensor("twob", (P, 4 * HT), f32, kind="ExternalInput")
    d2d = nc.dram_tensor("d2", (P, 2 * HT), f32, kind="ExternalInput")
    l4cd = nc.dram_tensor("l4c", (A, 3), f32, kind="ExternalInput")  # thr|twob|u0
    outd = nc.dram_tensor("out", (A, BS), f16, kind="ExternalOutput")

    with tile.TileContext(nc) as tc:
        with (
            tc.tile_pool(name="const", bufs=1) as cp,
            tc.tile_pool(name="state", bufs=1) as stp,
            tc.tile_pool(name="xp", bufs=2) as xp,
            tc.tile_pool(name="rp", bufs=2) as rp,
            tc.tile_pool(name="wv", bufs=2) as wvp,
            tc.tile_pool(name="l4t", bufs=1) as l4p,
            tc.tile_pool(name="ps", bufs=3, space="PSUM") as pp,
            tc.tile_pool(name="ps4", bufs=1, space="PSUM") as pp4,
        ):
            # ---- resident weights / constants ----
            w1sb = cp.tile([P, 2, KT1, H], f16)
            nc.sync.dma_start(
                w1sb[:], w1d.ap().rearrange("p (c k h) -> p c k h", c=2, k=KT1)
            )
            w2sb = cp.tile([P, 2, KT, H], f16)
            nc.sync.dma_start(
                w2sb[:], w2d.ap().rearrange("p (c k h) -> p c k h", c=2, k=KT)
            )
            w3sb = cp.tile([P, KT, H], f16)
            nc.sync.dma_start(
                w3sb[:], w3d.ap().rearrange("p (k h) -> p k h", k=KT)
            )
            w4sb = cp.tile([P, KT, A], f16)
            nc.sync.dma_start(
                w4sb[:], w4d.ap().rearrange("p (k a) -> p k a", k=KT)
            )
            thrsb = cp.tile([P, 4 * HT], f32)
            nc.sync.dma_start(thrsb[:], thrd.ap())
            twobsb = cp.tile([P, 4 * HT], f32)
            nc.sync.dma_start(twobsb[:], twobd.ap())
            d2sb = cp.tile([P, 2 * HT], f32)
            nc.sync.dma_start(d2sb[:], d2d.ap())
            l4c = cp.tile([A, 3], f32)
            nc.sync.dma_start(l4c[:], l4cd.ap())

            # ---- states ----
            u_st = [stp.tile([P, HT * NB], f32, tag=f"u{l}", name=f"u{l}")
                    for l in range(3)]
            vr_st = [stp.tile([P, HT * NB], f32, tag=f"vr{l}", name=f"vr{l}")
                     for l in range(3)]
            for l in range(3):
                nc.sync.dma_start(u_st[l][:], u0d.ap()[l])
                nc.vector.memset(vr_st[l][:], 0.0)
            u4 = stp.tile([A, NB], f16, tag="u4")
            vr4 = stp.tile([A, NB], f16, tag="vr4")
            acc = stp.tile([A, NB], f16, tag="acc")
            nc.vector.memset(u4[:], 0.0)
            nc.vector.tensor_scalar(u4[:], u4[:], l4c[:, 2:3], None, op0=AOT.add)
            nc.vector.memset(vr4[:], 0.0)
            nc.vector.memset(acc[:], 0.0)

            pending_op4 = []

            def flush_op4():
                while pending_op4:
                    vr_sl, (wv_, hb), twob_ap, r_ap = pending_op4.pop(0)
                    nc.vector.scalar_tensor_tensor(
                        vr_sl, wv_[:, hb], twob_ap, r_ap,
                        op0=AOT.add, op1=AOT.mult,
                    )

            def neuron_pair(l, jp, ps2, r_tile, ccol, skip_op4=False):
                """Shifted-state neuron update for an htile PAIR (j0, j0+1).
                Spikes r are emitted pre-scaled by VDECAY (stationaries carry
                1/VDECAY), so the vr state holds vd = VDECAY*vr and op2 is a
                plain TT add (GpSimd-legal).  op1/op2 process the pair in one
                1024-wide op; op3/op4 are per-htile (per-feature scalars).
                op4 deferred one pair."""
                j0 = 2 * jp
                sl2 = slice(j0 * NB, (j0 + 2) * NB)
                u_sl2 = u_st[l][:, sl2]
                vr_sl2 = vr_st[l][:, sl2]
                nc.vector.scalar_tensor_tensor(
                    u_sl2, u_sl2, CDECAY, ps2[:], op0=AOT.mult, op1=AOT.add
                )
                wv2 = wvp.tile([P, 2 * NB], f32, tag="wv")
                eng2 = nc.vector if jp in OP2_DVE else nc.gpsimd
                eng2.tensor_tensor(wv2[:], vr_sl2, u_sl2, op=AOT.add)
                for h in range(2):
                    j = j0 + h
                    cj = ccol + j
                    hb = slice(h * NB, (h + 1) * NB)
                    nc.gpsimd.tensor_scalar(
                        r_tile[:, j, :], wv2[:, hb], thrsb[:, cj : cj + 1],
                        VDECAY, op0=AOT.is_le, op1=AOT.mult,
                    )
                flush_op4()
                if not skip_op4:
                    for h in range(2):
                        j = j0 + h
                        cj = ccol + j
                        hb = slice(h * NB, (h + 1) * NB)
                        pending_op4.append(
                            (vr_st[l][:, slice(j * NB, (j + 1) * NB)],
                             (wv2, hb), twobsb[:, cj : cj + 1], r_tile[:, j, :])
                        )

            mm = nc.tensor.matmul

            def l1_block(t, last=False):
                """Layer 1 for step t: emitted one step ahead so the PE has
                dependency-free work to overlap the previous step's tail."""
                xt = xp.tile([P, 2, KT1, NB], f16, tag="xt", name="xt")
                nc.sync.dma_start(
                    xt[:], xd.ap()[t].rearrange("p (c k b) -> p c k b", c=2, k=KT1)
                )
                r1 = rp.tile([P, KT, NB], f16, tag="r", name="r1")
                for jp in range(HT // 2):
                    ps2 = pp.tile([P, 2 * NB], f32, tag="ps", name="ps")
                    for h in range(2):
                        j = 2 * jp + h
                        hs = slice(j * P, (j + 1) * P)
                        hb = slice(h * NB, (h + 1) * NB)
                        for k in range(KT1):
                            mm(ps2[:, hb], w1sb[:, 0, k, hs], xt[:, 0, k, :],
                               start=(k == 0), stop=False)
                            mm(ps2[:, hb], w1sb[:, 0, k, hs], xt[:, 1, k, :],
                               start=False, stop=False)
                        for k in range(KT1):
                            mm(ps2[:, hb], w1sb[:, 1, k, hs], xt[:, 0, k, :],
                               start=False, stop=(k == KT1 - 1))
                    neuron_pair(0, jp, ps2, r1, 0, skip_op4=last)
                return r1

            def rebase_u2(sign_col):
                """u2 += d2 (sign_col 0) or -= (sign_col 1) at phase switch."""
                for j in range(HT):
                    sl = slice(j * NB, (j + 1) * NB)
                    c = sign_col * HT + j
                    nc.vector.tensor_scalar(
                        u_st[1][:, sl], u_st[1][:, sl], d2sb[:, c : c + 1],
                        None, op0=AOT.add,
                    )

            tlist = [tt for _ in range(REPEAT) for tt in range(T)]
            r_l1 = l1_block(tlist[0])
            for ti, t in enumerate(tlist):
                early2 = t < T2SPLIT
                last_step = ti == len(tlist) - 1
                if ti > 0 and t == 0:
                    rebase_u2(1)       # REPEAT wrap: back to early constants
                if t == T2SPLIT:
                    rebase_u2(0)
                # ---- layer 2 (hi+res early, hi late) ----
                r_prev = r_l1
                r2 = rp.tile([P, KT, NB], f16, tag="r")
                for jp in range(HT // 2):
                    ps2 = pp.tile([P, 2 * NB], f32, tag="ps")
                    for h in range(2):
                        j = 2 * jp + h
                        hs = slice(j * P, (j + 1) * P)
                        hb = slice(h * NB, (h + 1) * NB)
                        for k in range(KT):
                            mm(ps2[:, hb], w2sb[:, 0, k, hs], r_prev[:, k, :],
                               start=(k == 0),
                               stop=(not early2 and k == KT - 1))
                        if early2:
                            for k in range(KT):
                                mm(ps2[:, hb], w2sb[:, 1, k, hs], r_prev[:, k, :],
                                   start=False, stop=(k == KT - 1))
                    neuron_pair(1, jp, ps2, r2, HT if early2 else 2 * HT,
                                skip_op4=last_step)
                # ---- layer 3 (hi only) ----
                r3 = rp.tile([P, KT, NB], f16, tag="r")
                for jp in range(HT // 2):
                    ps2 = pp.tile([P, 2 * NB], f32, tag="ps")
                    for h in range(2):
                        j = 2 * jp + h
                        hs = slice(j * P, (j + 1) * P)
                        hb = slice(h * NB, (h + 1) * NB)
                        for k in range(KT):
                            mm(ps2[:, hb], w3sb[:, k, hs], r2[:, k, :],
                               start=(k == 0), stop=(k == KT - 1))
                    neuron_pair(2, jp, ps2, r3, 3 * HT, skip_op4=last_step)
                # ---- layer 1 of next step (software pipeline) ----
                if ti + 1 < len(tlist):
                    r_l1 = l1_block(tlist[ti + 1], last=(ti + 2 == len(tlist)))
                # ---- layer 4 (hi only) ----
                ps4 = pp4.tile([A, NB], f32, tag="ps4")
                for k in range(KT):
                    mm(ps4[:], w4sb[:, k, :], r3[:, k, :],
                       start=(k == 0), stop=(k == KT - 1))
                flush_op4()
                nc.vector.scalar_tensor_tensor(
                    u4[:], u4[:], CDECAY, ps4[:], op0=AOT.mult, op1=AOT.add
                )
                wv4 = l4p.tile([A, NB], f16, tag="wv4")
                nc.vector.scalar_tensor_tensor(
                    wv4[:], vr4[:], VDECAY, u4[:], op0=AOT.mult, op1=AOT.add
                )
                r4 = l4p.tile([A, NB], f16, tag="r4")
                nc.gpsimd.tensor_scalar(
                    r4[:], wv4[:], l4c[:, 0:1], None, op0=AOT.is_le
                )
                nc.vector.scalar_tensor_tensor(
                    vr4[:], wv4[:], l4c[:, 1:2], r4[:], op0=AOT.add, op1=AOT.mult
                )
                nc.vector.scalar_tensor_tensor(
                    acc[:], acc[:], 1.0, r4[:], op0=AOT.add, op1=AOT.subtract
                )

            nc.sync.dma_start(outd.ap(), acc[:])

    nc.compile()
    return nc


def _prep_shared(W1, b1, W2, b2, W3, b3, W4, b4):
    """Host-side weight/constant prep shared by all cores."""
    w1hi, w1res = _f16pair(np.ascontiguousarray(W1.T))  # [S, H]
    w1t = np.empty((P, 2, KT1, H), np.float16)
    w1t[:, 0] = np.transpose(w1hi.reshape(KT1, P, H), (1, 0, 2))
    w1t[:, 1] = np.transpose(w1res.reshape(KT1, P, H), (1, 0, 2))
    w1t = np.ascontiguousarray(w1t.reshape(P, 2 * KT1 * H))

    w2hi, w2res = _f16pair(np.ascontiguousarray((-W2).T / VDECAY))  # [K, H]
    w2t = np.empty((P, 2, KT, H), np.float16)
    w2t[:, 0] = np.transpose(w2hi.reshape(KT, P, H), (1, 0, 2))
    w2t[:, 1] = np.transpose(w2res.reshape(KT, P, H), (1, 0, 2))
    w2t = np.ascontiguousarray(w2t.reshape(P, 2 * KT * H))

    w3hi, _ = _f16pair(np.ascontiguousarray((-W3).T / VDECAY))
    w3t = np.ascontiguousarray(
        np.transpose(w3hi.reshape(KT, P, H), (1, 0, 2)).reshape(P, KT * H)
    )
    w4hi, _ = _f16pair(np.ascontiguousarray((-W4).T / VDECAY))  # [K, A]
    w4t = np.ascontiguousarray(
        np.transpose(w4hi.reshape(KT, P, A), (1, 0, 2)).reshape(P, KT * A)
    )

    # b_eff per layer/phase, from the EFFECTIVE on-chip weights
    f64 = np.float64
    # spikes are emitted pre-scaled by VDECAY, stationaries carry 1/VDECAY;
    # the effective weight is VDECAY * (on-chip hi/res sum).
    VD64 = f64(VDECAY)
    be1 = b1.astype(f64)                      # L1: plain W1 (no complement)
    w2eff_E = w2hi.astype(f64) + w2res.astype(f64)   # = -(W2eff).T / VD
    be2E = b2.astype(f64) - VD64 * w2eff_E.sum(axis=0)
    be2L = b2.astype(f64) - VD64 * w2hi.astype(f64).sum(axis=0)
    be3 = b3.astype(f64) - VD64 * w3hi.astype(f64).sum(axis=0)
    be4 = b4.astype(f64) - VD64 * w4hi.astype(f64).sum(axis=0)

    # shifted-form constants, columns [L1 | L2E | L2L | L3], feature h=j*P+p
    thr = np.empty((P, 4 * HT), np.float32)
    twob = np.empty((P, 4 * HT), np.float32)
    u0 = np.empty((3, P, HT * NB), np.float32)
    col_bes = [be1, be2E, be2L, be3]
    for c, bev in enumerate(col_bes):
        for j in range(HT):
            fv = bev[j * P : (j + 1) * P]
            thr[:, c * HT + j] = (VTH - 2.0 * fv).astype(np.float32)
            twob[:, c * HT + j] = (2.0 * fv).astype(np.float32)
    for l, bev in enumerate([be1, be2E, be3]):
        for j in range(HT):
            fv = bev[j * P : (j + 1) * P]
            u0[l, :, j * NB : (j + 1) * NB] = np.broadcast_to(
                (-2.0 * fv).astype(np.float32)[:, None], (P, NB)
            )
    # u2 rebase at L2 phase switch: u'_L = u'_E + 2(beE - beL)
    d2 = np.empty((P, 2 * HT), np.float32)
    for j in range(HT):
        dv = (2.0 * (be2E - be2L))[j * P : (j + 1) * P]
        d2[:, j] = dv.astype(np.float32)
        d2[:, HT + j] = (-dv).astype(np.float32)

    l4c = np.stack(
        [
            (VTH - 2.0 * be4).astype(np.float32),
            (2.0 * be4).astype(np.float32),
            (-2.0 * be4).astype(np.float32),
        ],
        axis=1,
    )  # [A, 3]
    return dict(w1=w1t, w2=w2t, w3=w3t, w4=w4t,
                thr=np.ascontiguousarray(thr), twob=np.ascontiguousarray(twob),
                d2=np.ascontiguousarray(d2),
                u0=np.ascontiguousarray(u0), l4c=np.ascontiguousarray(l4c))


def _prep_x_core(xc):
    """xc [BS, S, T'] fp32 -> [T', P, 2*KT1*NB] fp16 (hi|res, partition-major)."""
    Tc = xc.shape[2]
    xt = np.transpose(xc, (2, 1, 0)).astype(np.float32)  # [T', S, BS]
    hi = xt.astype(np.float16)
    res = (xt - hi.astype(np.float32)).astype(np.float16)
    out = np.empty((Tc, P, 2, KT1, NB), np.float16)
    for c, arr in ((0, hi), (1, res)):
        out[:, :, c, :, :] = np.transpose(arr.reshape(Tc, KT1, P, NB), (0, 2, 1, 3))
    return np.ascontiguousarray(out.reshape(Tc, P, 2 * KT1 * NB))


def _get_nc():
    if "nc" not in _CACHE:
        _CACHE["nc"] = _build_program()
    return _CACHE["nc"]


def kernel(x, W1, b1, W2, b2, W3, b3, W4, b4, batch_size, _trace=False):
    from concourse.bass_utils import run_bass_kernel_spmd

    x = np.asarray(x, np.float32)
    W1, b1 = np.asarray(W1, np.float32), np.asarray(b1, np.float32)
    W2, b2 = np.asarray(W2, np.float32), np.asarray(b2, np.float32)
    W3, b3 = np.asarray(W3, np.float32), np.asarray(b3, np.float32)
    W4, b4 = np.asarray(W4, np.float32), np.asarray(b4, np.float32)
    assert x.shape == (B, S, T)

    nc = _get_nc()
    shared = _prep_shared(W1, b1, W2, b2, W3, b3, W4, b4)
    in_maps = []
    for c in range(NCORES):
        m = dict(shared)
        m["x"] = _prep_x_core(x[c * BS : (c + 1) * BS])
        in_maps.append(m)

    res = run_bass_kernel_spmd(
        nc, in_maps, core_ids=list(range(NCORES)), trace=_trace
    )
    _CACHE["last_results"] = res
    out = np.empty((B, A), np.float32)
    for c in range(NCORES):
        out[c * BS : (c + 1) * BS] = res.results[c]["out"].T.astype(np.float32)
    return out / np.float32(T)
